# revision 1
# baseline (speedup 1.0000x reference)
"""MoE inverted-dispatch expert bank kernel for 8x Trainium2 NeuronCores.

Strategy (expert parallelism):
  - Host: replicate the reference routing (stable argsort -> per-expert rank,
    capacity drop), build a transposed per-expert token buffer bufT[E, d, C],
    cast weights + tokens to bf16, shard 8 experts per core.
  - Device (SPMD, 8 cores): per expert, grouped 2-layer MLP with the weights
    as the stationary matmul operand and the token buffer moving in
    [feature, token] layout:  hT[f, C] = gelu(W1.T-tiles @ bufT),
    yT[d, C] = W2-tiles @ hT.  fp32 PSUM accumulation, GELU on ScalarE.
  - Host: gather yT, scatter-combine to [N, k, d], loads = counts / N.
"""

import os
import numpy as np
import ml_dtypes

N_TOK = 4096
K_ACT = 2
D_MODEL = 512
D_FF = 2048
N_EXP = 64
CAP = 256
N_CORES = 8
E_PER = N_EXP // N_CORES  # 8 experts per core

P = 128
KT1 = D_MODEL // P   # 4  k-tiles for layer 1
MT1 = D_FF // P      # 16 m-tiles for layer 1
KT2 = D_FF // P      # 16 k-tiles for layer 2
MT2 = D_MODEL // P   # 4  m-tiles for layer 2

LAST_EXEC_TIME_NS = None

_CACHE = {}


def _install_trace_hook():
    """Register the axon NTFF profile hook under antenv.axon_hooks if the
    image's antenv package lacks it (needed for trace=True under axon)."""
    try:
        from antenv.axon_hooks import get_axon_ntff_profile_hook  # noqa: F401
        return True
    except ImportError:
        pass
    try:
        import sys, types
        import trn_agent_boot.trn_boot as tb
        hook = tb._ntff_profile_via_ctypes('/opt/axon/libaxon_pjrt.so')
        mod = types.ModuleType("antenv.axon_hooks")
        mod.get_axon_ntff_profile_hook = lambda: hook
        mod.set_axon_ntff_profile_hook = lambda h: None
        sys.modules['antenv.axon_hooks'] = mod
        import antenv
        antenv.axon_hooks = mod
        return True
    except Exception:
        return False


def _build_kernel():
    import concourse.tile as tile
    import concourse.mybir as mybir
    from concourse import bacc
    from concourse.bass import ts

    bf16 = mybir.dt.bfloat16
    f32 = mybir.dt.float32

    nc = bacc.Bacc("TRN2", target_bir_lowering=False, debug=False)
    bufT = nc.dram_tensor("bufT", [E_PER, D_MODEL, CAP], bf16, kind="ExternalInput")
    w1 = nc.dram_tensor("w1", [E_PER, D_MODEL, D_FF], bf16, kind="ExternalInput")
    w2 = nc.dram_tensor("w2", [E_PER, D_FF, D_MODEL], bf16, kind="ExternalInput")
    yT = nc.dram_tensor("yT", [E_PER, D_MODEL, CAP], f32, kind="ExternalOutput")

    with tile.TileContext(nc) as tc:
        with (
            tc.tile_pool(name="xpool", bufs=E_PER) as xpool,
            tc.tile_pool(name="w1pool", bufs=3) as w1pool,
            tc.tile_pool(name="w2pool", bufs=3) as w2pool,
            tc.tile_pool(name="hpool", bufs=2) as hpool,
            tc.tile_pool(name="ypool", bufs=4) as ypool,
            tc.tile_pool(name="ps1", bufs=4, space="PSUM") as ps1pool,
            tc.tile_pool(name="ps2", bufs=4, space="PSUM") as ps2pool,
        ):
            # token buffers for all local experts stay resident
            xs = []
            for e in range(E_PER):
                xt = xpool.tile([P, KT1, CAP], bf16, tag="x")
                nc.sync.dma_start(xt[:], bufT[e].rearrange("(ko p) c -> p ko c", p=P))
                xs.append(xt)

            for e in range(E_PER):
                w1t = w1pool.tile([P, KT1, D_FF], bf16, tag="w1")
                nc.sync.dma_start(w1t[:], w1[e].rearrange("(ko p) f -> p ko f", p=P))
                w2t = w2pool.tile([P, KT2, D_MODEL], bf16, tag="w2")
                nc.sync.dma_start(w2t[:], w2[e].rearrange("(ko p) d -> p ko d", p=P))

                ht = hpool.tile([P, KT2, CAP], bf16, tag="h")
                for m in range(MT1):
                    ps = ps1pool.tile([P, CAP], f32, tag="ps1")
                    for k in range(KT1):
                        nc.tensor.matmul(
                            ps[:],
                            lhsT=w1t[:, k, ts(m, P)],
                            rhs=xs[e][:, k, :],
                            start=(k == 0),
                            stop=(k == KT1 - 1),
                        )
                    nc.scalar.activation(
                        ht[:, m, :], ps[:], mybir.ActivationFunctionType.Gelu
                    )

                for m in range(MT2):
                    ps = ps2pool.tile([P, CAP], f32, tag="ps2")
                    for k in range(KT2):
                        nc.tensor.matmul(
                            ps[:],
                            lhsT=w2t[:, k, ts(m, P)],
                            rhs=ht[:, k, :],
                            start=(k == 0),
                            stop=(k == KT2 - 1),
                        )
                    yt = ypool.tile([P, CAP], f32, tag="y")
                    nc.vector.tensor_copy(yt[:], ps[:])
                    nc.sync.dma_start(yT[e, ts(m, P), :], yt[:])

    nc.compile()
    return nc


def kernel(hidden_states, selected_experts, expert_masks, W1, W2):
    global LAST_EXEC_TIME_NS
    from concourse.bass_utils import run_bass_kernel_spmd

    hidden = np.ascontiguousarray(np.asarray(hidden_states, dtype=np.float32))
    sel = np.asarray(selected_experts).astype(np.int64)
    W1 = np.asarray(W1, dtype=np.float32)
    W2 = np.asarray(W2, dtype=np.float32)

    # ---- host dispatch (mirrors reference routing exactly) ----
    flat_e = sel.reshape(-1)
    S = flat_e.shape[0]
    order = np.argsort(flat_e, kind="stable")
    e_sorted = flat_e[order]
    counts = np.bincount(flat_e, minlength=N_EXP)
    offsets = np.cumsum(counts) - counts
    rank = np.arange(S, dtype=np.int64) - offsets[e_sorted]
    tok = order // K_ACT
    slot = order % K_ACT
    valid = rank < CAP
    ev, rv, tv, sv = e_sorted[valid], rank[valid], tok[valid], slot[valid]

    bf = ml_dtypes.bfloat16
    hidden16 = hidden.astype(bf)
    bufT = np.zeros((N_EXP, D_MODEL, CAP), dtype=bf)
    bufT[ev, :, rv] = hidden16[tv]
    W1_16 = W1.astype(bf)
    W2_16 = W2.astype(bf)

    # ---- device: grouped GEMMs, 8 experts per core ----
    trace = os.environ.get("KERNEL_TRACE", "0") == "1" and _install_trace_hook()

    if "nc" not in _CACHE:
        _CACHE["nc"] = _build_kernel()
    nc = _CACHE["nc"]

    in_maps = [
        {
            "bufT": np.ascontiguousarray(bufT[i * E_PER:(i + 1) * E_PER]),
            "w1": np.ascontiguousarray(W1_16[i * E_PER:(i + 1) * E_PER]),
            "w2": np.ascontiguousarray(W2_16[i * E_PER:(i + 1) * E_PER]),
        }
        for i in range(N_CORES)
    ]

    res = run_bass_kernel_spmd(
        nc, in_maps, core_ids=list(range(N_CORES)), trace=trace
    )
    LAST_EXEC_TIME_NS = res.exec_time_ns

    yT = np.concatenate(
        [res.results[i]["yT"] for i in range(N_CORES)], axis=0
    )  # [N_EXP, D_MODEL, CAP] fp32

    # ---- host combine ----
    gathered = yT[ev, :, rv]  # [Sv, D_MODEL]
    expert_outputs = np.zeros((N_TOK, K_ACT, D_MODEL), dtype=np.float32)
    expert_outputs[tv, sv] = gathered
    expert_loads = counts.astype(np.float32) / N_TOK
    return expert_outputs, expert_loads
